# revision 1
# baseline (speedup 1.0000x reference)
"""Confusion-matrix (150x150) histogram kernel for Trainium2, 8 NeuronCores.

Algorithm
---------
cm[t, p] += 1 for 8.4M (t, p) pairs == histogram over 22500 bins of
bin = t*150 + p.  Data-parallel over 8 cores (1M elements each).

On-device per core: no scatter exists on TRN2, so counting is done as a
one-hot outer-product accumulated by the tensor engine:

    bin = t*150 + p          (DVE, exact: products <= 22350)
    v   = bin & 127          (128-wide one-hot -> matmul lhsT)
    u   = bin >> 7           (176-wide one-hot -> matmul rhs)
    psum[v, u] += onehot(v)^T @ onehot(u)   (PSUM f32, exact integer adds)

One-hot supply is the bottleneck (2 TensorScalarPtr per 128-element chunk
on DVE, ~258ns/chunk: ~90ns instr overhead+scalar-load each).  To beat the
DVE floor, a fraction of chunks is routed through the otherwise-idle ACT
engine: Square(-iota + v) gives (iota-v)^2 (zero exactly at the match),
and one batched DVE is_equal(.,0) with an *immediate* scalar (cheap, 4x
mode) converts 4 chunks' squares to one-hots at ~94ns/chunk of DVE time.
DVE and ACT run concurrently; the split A_CHUNKS balances their loads.

Host unpacks psum[v, u] into counts[bin = u*128 + v] and sums the 8 cores.
"""

import numpy as np

NUM_CLASSES = 150
N = 8_388_608
N_CORES = 8
P = 128
PER_CORE = N // N_CORES          # 1_048_576
E = PER_CORE // P                # 8192 elements per partition row
TILE_E = 512                     # elements-per-partition per DMA/prep tile
NT = E // TILE_E                 # 16 tiles
VW = 128                         # v one-hot width (lhsT free / out partitions)
UW = 176                         # u one-hot width (rhs / out free dim)
CW = VW + UW                     # combined one-hot row for the ACT path
GACT = 4                         # chunks per batched ACT-path is_equal
A_CHUNKS = 0                     # ACT-path share; 0 = pure DVE path (fastest measured)

_cached_nc = {}


def _build_module(repeat=1, a_chunks=A_CHUNKS):
    global _cached_nc
    key = (repeat, a_chunks)
    if key in _cached_nc:
        return _cached_nc[key]
    from contextlib import ExitStack

    import concourse.bass as bass
    import concourse.tile as tile
    from concourse import bacc, mybir

    nc = bacc.Bacc(
        "TRN2",
        target_bir_lowering=False,
        debug=False,
        enable_asserts=False,
        num_devices=N_CORES,
    )
    t_d = nc.dram_tensor("t", [P, E], mybir.dt.int32, kind="ExternalInput")
    p_d = nc.dram_tensor("p", [P, E], mybir.dt.int32, kind="ExternalInput")
    out_d = nc.dram_tensor("out", [P, UW], mybir.dt.float32, kind="ExternalOutput")

    i32 = mybir.dt.int32
    bf16 = mybir.dt.bfloat16
    f32 = mybir.dt.float32
    Op = mybir.AluOpType
    Act = mybir.ActivationFunctionType

    n_mm_total = repeat * NT * TILE_E

    with tile.TileContext(nc) as tc, ExitStack() as ctx:
        const_pool = ctx.enter_context(tc.tile_pool(name="const", bufs=1))
        io_pool = ctx.enter_context(tc.tile_pool(name="io", bufs=3))
        prep_pool = ctx.enter_context(tc.tile_pool(name="prep", bufs=2))
        oh_pool = ctx.enter_context(tc.tile_pool(name="oh", bufs=16))
        blk_pool = ctx.enter_context(tc.tile_pool(name="blk", bufs=8))
        psum_pool = ctx.enter_context(tc.tile_pool(name="psum", bufs=1, space="PSUM"))

        iota_i = const_pool.tile([P, UW], i32)
        nc.gpsimd.iota(iota_i[:], pattern=[[1, UW]], base=0, channel_multiplier=0)
        iota_bf = const_pool.tile([P, UW], bf16)
        nc.vector.tensor_copy(iota_bf[:], iota_i[:])

        psum = psum_pool.tile([P, UW], f32)

        mm = 0

        def do_mm(lhsT, rhs):
            nonlocal mm
            nc.tensor.matmul(
                psum[:], lhsT, rhs, start=(mm == 0), stop=(mm == n_mm_total - 1)
            )
            mm += 1

        for _rep in range(repeat):
            for it in range(NT):
                t_t = io_pool.tile([P, TILE_E], i32, tag="tin")
                nc.sync.dma_start(t_t[:], t_d.ap()[:, bass.ts(it, TILE_E)])
                p_t = io_pool.tile([P, TILE_E], i32, tag="pin")
                nc.sync.dma_start(p_t[:], p_d.ap()[:, bass.ts(it, TILE_E)])

                # bin = t*150 + p  (int32, exact), then v/u as f32 scalars
                bin_t = prep_pool.tile([P, TILE_E], i32, tag="bin")
                nc.vector.scalar_tensor_tensor(
                    bin_t[:], t_t[:], 150, p_t[:], op0=Op.mult, op1=Op.add
                )
                u_i = prep_pool.tile([P, TILE_E], i32, tag="ui")
                nc.vector.tensor_scalar(
                    u_i[:], bin_t[:], 7, None, op0=Op.logical_shift_right
                )
                v_i = prep_pool.tile([P, TILE_E], i32, tag="vi")
                nc.vector.tensor_scalar(v_i[:], bin_t[:], 127, None, op0=Op.bitwise_and)
                u_sc = prep_pool.tile([P, TILE_E], f32, tag="ub")
                nc.vector.tensor_copy(u_sc[:], u_i[:])
                v_sc = prep_pool.tile([P, TILE_E], f32, tag="vb")
                nc.vector.tensor_copy(v_sc[:], v_i[:])

                # Interleave the ACT path (chunks [0, a_chunks), in groups
                # of GACT) with the DVE path (chunks [a_chunks, TILE_E)).
                # The batched is_equal + matmuls for an ACT group are emitted
                # one group *after* its Square instrs so DVE/PE never wait on
                # fresh ACT output; DVE chunks fill the gap in proportion.
                n_groups = a_chunks // GACT
                n_dve = TILE_E - a_chunks

                def emit_act_group(e0):
                    sq_blk = blk_pool.tile([P, GACT * CW], bf16, tag="sq")
                    for g in range(GACT):
                        e = e0 + g
                        nc.scalar.activation(
                            sq_blk[:, g * CW : g * CW + VW],
                            iota_bf[:, 0:VW],
                            Act.Square,
                            bias=v_sc[:, e : e + 1],
                            scale=-1.0,
                        )
                        nc.scalar.activation(
                            sq_blk[:, g * CW + VW : (g + 1) * CW],
                            iota_bf[:],
                            Act.Square,
                            bias=u_sc[:, e : e + 1],
                            scale=-1.0,
                        )
                    return sq_blk

                def drain_act_group(sq_blk):
                    oh_blk = blk_pool.tile([P, GACT * CW], bf16, tag="ohb")
                    nc.vector.tensor_scalar(
                        oh_blk[:], sq_blk[:], 0.0, None, op0=Op.is_equal
                    )
                    for g in range(GACT):
                        do_mm(
                            oh_blk[:, g * CW : g * CW + VW],
                            oh_blk[:, g * CW + VW : (g + 1) * CW],
                        )

                def emit_dve_chunk(e):
                    oh_v = oh_pool.tile([P, VW], bf16, tag="ohv")
                    nc.vector.tensor_scalar(
                        oh_v[:], iota_bf[:, 0:VW], v_sc[:, e : e + 1], None,
                        op0=Op.is_equal,
                    )
                    oh_u = oh_pool.tile([P, UW], bf16, tag="ohu")
                    nc.vector.tensor_scalar(
                        oh_u[:], iota_bf[:], u_sc[:, e : e + 1], None,
                        op0=Op.is_equal,
                    )
                    do_mm(oh_v[:], oh_u[:])

                pending = []
                dve_cursor = a_chunks
                for gi in range(n_groups):
                    pending.append(emit_act_group(gi * GACT))
                    if len(pending) > 2:
                        drain_act_group(pending.pop(0))
                    # proportional share of DVE chunks after this group
                    hi = a_chunks + ((gi + 1) * n_dve) // n_groups
                    while dve_cursor < hi:
                        emit_dve_chunk(dve_cursor)
                        dve_cursor += 1
                for sq in pending:
                    drain_act_group(sq)
                pending = []
                while dve_cursor < TILE_E:
                    emit_dve_chunk(dve_cursor)
                    dve_cursor += 1

        out_sb = const_pool.tile([P, UW], f32)
        nc.vector.tensor_copy(out_sb[:], psum[:])
        nc.sync.dma_start(out_d.ap()[:, :], out_sb[:])

    nc.compile()
    _cached_nc[key] = nc
    return nc


def _ensure_axon_hooks_stub():
    try:
        import antenv.axon_hooks  # noqa: F401
    except ImportError:
        import sys
        import types

        mod = types.ModuleType("antenv.axon_hooks")
        mod.get_axon_ntff_profile_hook = lambda: None
        sys.modules["antenv.axon_hooks"] = mod


def kernel(confusion_matrix, predictions, targets):
    from concourse import bass_utils

    _ensure_axon_hooks_stub()

    preds = np.ascontiguousarray(np.asarray(predictions).astype(np.int32))
    targs = np.ascontiguousarray(np.asarray(targets).astype(np.int32))
    cm_in = np.asarray(confusion_matrix, dtype=np.float32)
    assert preds.shape == (N,) and targs.shape == (N,)

    nc = _build_module()

    in_maps = []
    for c in range(N_CORES):
        sl = slice(c * PER_CORE, (c + 1) * PER_CORE)
        in_maps.append(
            {
                "t": targs[sl].reshape(P, E),
                "p": preds[sl].reshape(P, E),
            }
        )

    res = bass_utils.run_bass_kernel_spmd(
        nc, in_maps, core_ids=list(range(N_CORES))
    )
    global _last_results, _last_nc
    _last_results = res
    _last_nc = nc

    counts = np.zeros((NUM_CLASSES * NUM_CLASSES,), dtype=np.float64)
    for c in range(N_CORES):
        part = res.results[c]["out"]          # [VW=128, UW=176], part[v, u]
        flat = part.T.reshape(-1)             # index u*128 + v == bin
        counts += flat[: NUM_CLASSES * NUM_CLASSES].astype(np.float64)

    out = cm_in + counts.reshape(NUM_CLASSES, NUM_CLASSES).astype(np.float32)
    return out.astype(np.float32)



# revision 2
# speedup vs baseline: 1.3626x; 1.3626x over previous
"""Confusion-matrix (150x150) histogram kernel for Trainium2, 8 NeuronCores.

cm[t, p] += 1 for 8.4M (t, p) pairs == histogram over 22500 bins of
bin = t*150 + p. Data-parallel over 8 cores (1M elements each); the host
splits bin into v = bin & 127 and u = bin >> 7 and uploads both as f32
[128, 8192] per core; the 8 partial [128, 176] count matrices are summed
on the host (the all-reduce of a 90KB tensor is cheaper off-device).

On-device per core there is no scatter, so counting is a one-hot
outer-product accumulated by the tensor engine:

    oh_v[p, f] = (iota128[f] == v[p, e])     DVE tensor_scalar ptr (2x mode)
    oh_u[p, f] = (iota176[f] == u[p, e])     DVE tensor_scalar ptr (2x mode)
    psum[v, u] += oh_v^T @ oh_u              PE matmul (f32 PSUM, exact)

The DVE one-hot supply is the wall (~265ns per 128-element chunk; the
per-partition scalar read occupies the second SBUF port, capping the op at
2x mode). A tuned fraction of chunks is therefore routed through the
otherwise-idle ACT engine: Square(-iota + v) gives (iota - v)^2 (zero
exactly at the match) and one batched DVE is_equal(., 0) with an immediate
scalar (true 4x mode) converts a group of chunks' squares to one-hots at
~94ns/chunk of DVE time. DVE and ACT run concurrently; A_CHUNKS per
512-chunk tile balances their loads (ACT saturates ~500ns/chunk).
"""

import numpy as np

NUM_CLASSES = 150
N = 8_388_608
N_CORES = 8
P = 128
PER_CORE = N // N_CORES          # 1_048_576
E = PER_CORE // P                # 8192 chunks per core
TILE_E = 512                     # chunks per DMA tile
NT = E // TILE_E                 # 16 tiles
VW = 128
UW = 176
CW = VW + UW

A_CHUNKS = 160                   # ACT-path chunks per tile (of TILE_E)
GACT = 8                         # chunks per batched ACT-path is_equal
PEND = 6                         # ACT groups in flight before draining
OH_BUFS = 64

_cached = {}


def _build_module(repeat=1, a_chunks=A_CHUNKS, oh_bufs=OH_BUFS, pend=PEND,
                  gact=GACT):
    key = (repeat, a_chunks, oh_bufs, pend, gact)
    if key in _cached:
        return _cached[key]
    from contextlib import ExitStack

    import concourse.bass as bass
    import concourse.tile as tile
    from concourse import bacc, mybir

    nc = bacc.Bacc(
        "TRN2",
        target_bir_lowering=False,
        debug=False,
        enable_asserts=False,
        num_devices=N_CORES,
    )
    f32 = mybir.dt.float32
    bf16 = mybir.dt.bfloat16
    i32 = mybir.dt.int32
    Op = mybir.AluOpType
    Act = mybir.ActivationFunctionType

    v_d = nc.dram_tensor("v", [P, E], f32, kind="ExternalInput")
    u_d = nc.dram_tensor("u", [P, E], f32, kind="ExternalInput")
    out_d = nc.dram_tensor("out", [P, UW], f32, kind="ExternalOutput")

    n_mm_total = repeat * NT * TILE_E

    with tile.TileContext(nc) as tc, ExitStack() as ctx:
        const_pool = ctx.enter_context(tc.tile_pool(name="const", bufs=1))
        io_pool = ctx.enter_context(tc.tile_pool(name="io", bufs=3))
        oh_pool = ctx.enter_context(tc.tile_pool(name="oh", bufs=oh_bufs))
        sq_pool = ctx.enter_context(tc.tile_pool(name="sq", bufs=pend + 2))
        ohb_pool = ctx.enter_context(tc.tile_pool(name="ohb", bufs=4))
        psum_pool = ctx.enter_context(tc.tile_pool(name="psum", bufs=1, space="PSUM"))

        iota_i = const_pool.tile([P, UW], i32)
        nc.gpsimd.iota(iota_i[:], pattern=[[1, UW]], base=0, channel_multiplier=0)
        iota_bf = const_pool.tile([P, UW], bf16)
        nc.vector.tensor_copy(iota_bf[:], iota_i[:])

        psum = psum_pool.tile([P, UW], f32)
        mm = 0

        def do_mm(lhsT, rhs):
            nonlocal mm
            nc.tensor.matmul(
                psum[:], lhsT, rhs, start=(mm == 0), stop=(mm == n_mm_total - 1)
            )
            mm += 1

        for _rep in range(repeat):
            for it in range(NT):
                v_t = io_pool.tile([P, TILE_E], f32, tag="vin")
                nc.sync.dma_start(v_t[:], v_d.ap()[:, bass.ts(it, TILE_E)])
                u_t = io_pool.tile([P, TILE_E], f32, tag="uin")
                nc.sync.dma_start(u_t[:], u_d.ap()[:, bass.ts(it, TILE_E)])

                n_groups = a_chunks // gact
                n_dve = TILE_E - a_chunks

                def emit_act_group(e0):
                    sq_blk = sq_pool.tile([P, gact * CW], bf16, tag="sq")
                    for g in range(gact):
                        e = e0 + g
                        nc.scalar.activation(
                            sq_blk[:, g * CW : g * CW + VW],
                            iota_bf[:, 0:VW],
                            Act.Square,
                            bias=v_t[:, e : e + 1],
                            scale=-1.0,
                        )
                        nc.scalar.activation(
                            sq_blk[:, g * CW + VW : (g + 1) * CW],
                            iota_bf[:],
                            Act.Square,
                            bias=u_t[:, e : e + 1],
                            scale=-1.0,
                        )
                    return sq_blk

                def drain_act_group(sq_blk):
                    oh_blk = ohb_pool.tile([P, gact * CW], bf16, tag="ohb")
                    nc.vector.tensor_scalar(
                        oh_blk[:], sq_blk[:], 0.0, None, op0=Op.is_equal
                    )
                    for g in range(gact):
                        do_mm(
                            oh_blk[:, g * CW : g * CW + VW],
                            oh_blk[:, g * CW + VW : (g + 1) * CW],
                        )

                def emit_dve_chunk(e):
                    oh_v = oh_pool.tile([P, VW], bf16, tag="ohv")
                    nc.vector.tensor_scalar(
                        oh_v[:], iota_bf[:, 0:VW], v_t[:, e : e + 1], None,
                        op0=Op.is_equal,
                    )
                    oh_u = oh_pool.tile([P, UW], bf16, tag="ohu")
                    nc.vector.tensor_scalar(
                        oh_u[:], iota_bf[:], u_t[:, e : e + 1], None,
                        op0=Op.is_equal,
                    )
                    do_mm(oh_v[:], oh_u[:])

                pending = []
                dve_cursor = a_chunks
                for gi in range(n_groups):
                    pending.append(emit_act_group(gi * gact))
                    if len(pending) > pend:
                        drain_act_group(pending.pop(0))
                    hi = a_chunks + ((gi + 1) * n_dve) // max(n_groups, 1)
                    while dve_cursor < hi:
                        emit_dve_chunk(dve_cursor)
                        dve_cursor += 1
                for sq in pending:
                    drain_act_group(sq)
                while dve_cursor < TILE_E:
                    emit_dve_chunk(dve_cursor)
                    dve_cursor += 1

        out_sb = const_pool.tile([P, UW], f32)
        nc.vector.tensor_copy(out_sb[:], psum[:])
        nc.sync.dma_start(out_d.ap()[:, :], out_sb[:])

    nc.compile()
    _cached[key] = nc
    return nc


def _ensure_axon_hooks_stub():
    try:
        import antenv.axon_hooks  # noqa: F401
    except ImportError:
        import sys
        import types

        mod = types.ModuleType("antenv.axon_hooks")
        mod.get_axon_ntff_profile_hook = lambda: None
        sys.modules["antenv.axon_hooks"] = mod


def host_prep(predictions, targets):
    preds = np.asarray(predictions).astype(np.int64)
    targs = np.asarray(targets).astype(np.int64)
    bins = targs * NUM_CLASSES + preds
    v = (bins & 127).astype(np.float32)
    u = (bins >> 7).astype(np.float32)
    per_core = []
    for c in range(N_CORES):
        sl = slice(c * PER_CORE, (c + 1) * PER_CORE)
        per_core.append({"v": v[sl].reshape(P, E), "u": u[sl].reshape(P, E)})
    return per_core


def unpack(results, confusion_matrix):
    counts = np.zeros((NUM_CLASSES * NUM_CLASSES,), dtype=np.float64)
    for c in range(N_CORES):
        part = results[c]["out"]              # [VW, UW], part[v, u]
        flat = part.T.reshape(-1)             # index u*128 + v == bin
        counts += flat[: NUM_CLASSES * NUM_CLASSES].astype(np.float64)
    cm_in = np.asarray(confusion_matrix, dtype=np.float32)
    out = cm_in + counts.reshape(NUM_CLASSES, NUM_CLASSES).astype(np.float32)
    return out.astype(np.float32)


def kernel(confusion_matrix, predictions, targets):
    from concourse import bass_utils

    _ensure_axon_hooks_stub()
    assert np.asarray(predictions).shape == (N,)
    assert np.asarray(targets).shape == (N,)
    nc = _build_module()
    in_maps = host_prep(predictions, targets)
    res = bass_utils.run_bass_kernel_spmd(
        nc, in_maps, core_ids=list(range(N_CORES))
    )
    global _last_results, _last_nc
    _last_results = res
    _last_nc = nc
    return unpack(res.results, confusion_matrix)
